# revision 1
# baseline (speedup 1.0000x reference)
"""Trainium2 Bass kernel for CrossAttention (nn_CrossAttention_82343112999000).

Reference computation (per batch b):
  q = x @ Wq.T; k = ctx @ Wk.T; v = ctx @ Wv.T     (nn.Linear, W stored [out, in])
  per head: attn = softmax(q k^T / sqrt(hd)); o = attn @ v
  out = concat_heads(o) @ Wo.T + bo + x

Sharding: pure data parallel over the 4096 flattened query rows.
Core c handles batch b = c//4 and query rows [(c%4)*512, (c%4+1)*512).
Each core computes the full k/v for its batch (duplicated work across the 4
cores of a batch, but no collectives are needed).

On-chip dataflow (per core, all matmuls bf16 with fp32 PSUM accumulation):
  - inputs are cast fp32->bf16 during the SWDGE DMA (free), then transposed
    on-chip via PE-transpose into d-major layouts (xT, ctxT, WqT/WkT/WvT/WoT)
  - projections produce qT [e, m], kT [e, c] and v natural [c, e] directly
  - scoresT[c, m] = kT_h^T-tile @ qT_h  (contraction over hd=64; the two heads
    sharing a 128-partition tile run concurrently in separate PE row groups)
  - P = exp(scale * scoresT) on ACT, PSUM -> SBUF bf16
  - attn@v: lhsT = v augmented with a ones column -> attnoutT[hd, m] plus the
    softmax denominator row for free; normalize with DVE reciprocal+mul
  - out proj: lhsT = attnoutT tiles, rhs = WoT; + x (fp32) + bo on DVE
"""

import numpy as np

import concourse.bass as bass
import concourse.tile as tile
from concourse import bacc, mybir
from concourse.bass_utils import run_bass_kernel_spmd

f32 = mybir.dt.float32
bf16 = mybir.dt.bfloat16
Exp = mybir.ActivationFunctionType.Exp

B, L, LC, D, CD, H, HD = 2, 2048, 1024, 1024, 768, 16, 64
NCORES = 8
M = (B * L) // NCORES  # 512 query rows per core
MT = M // 128  # 4
DT = D // 128  # 8
CDT = CD // 128  # 6
CT = LC // 128  # 8
ET = D // 128  # 8
SCALE = float(HD) ** -0.5

LAST_RESULT = None  # BassKernelResults of the most recent run (for test.py)
_cached_nc = None


def _build():
    nc = bacc.Bacc("TRN2", target_bir_lowering=False, debug=False, num_devices=NCORES)
    x_d = nc.dram_tensor("x", [M, D], f32, kind="ExternalInput").ap()
    ctx_d = nc.dram_tensor("ctx", [LC, CD], f32, kind="ExternalInput").ap()
    wq_d = nc.dram_tensor("wq", [D, D], f32, kind="ExternalInput").ap()
    wk_d = nc.dram_tensor("wk", [D, CD], f32, kind="ExternalInput").ap()
    wv_d = nc.dram_tensor("wv", [D, CD], f32, kind="ExternalInput").ap()
    wo_d = nc.dram_tensor("wo", [D, D], f32, kind="ExternalInput").ap()
    bo_d = nc.dram_tensor("bo", [1, D], f32, kind="ExternalInput").ap()
    out_d = nc.dram_tensor("out", [M, D], f32, kind="ExternalOutput").ap()

    with tile.TileContext(nc) as tc:
        with (
            tc.tile_pool(name="const", bufs=1) as const_pool,
            tc.tile_pool(name="xf", bufs=1) as xf_pool,
            tc.tile_pool(name="persist", bufs=1) as persist,
            tc.tile_pool(name="p", bufs=6) as p_pool,
            tc.tile_pool(name="r", bufs=4) as r_pool,
            tc.tile_pool(name="outsb", bufs=2) as out_pool,
            tc.tile_pool(name="dram", bufs=3, space="DRAM") as dram_pool,
            tc.tile_pool(name="mmps", bufs=2, space="PSUM") as mmps,
            tc.tile_pool(name="scps", bufs=2, space="PSUM") as scps,
            tc.tile_pool(name="avps", bufs=2, space="PSUM") as avps,
        ):
            bo_sb = const_pool.tile([1, D], f32, tag="bo")
            nc.sync.dma_start(bo_sb[:], bo_d)
            bo_b = const_pool.tile([128, D], f32, tag="bo_b")
            nc.gpsimd.partition_broadcast(bo_b[:], bo_sb[:])

            # persistent transposed/derived tensors
            xT = persist.tile([128, DT, M], bf16, tag="xT")
            cT = persist.tile([128, CDT, LC], bf16, tag="cT")
            wqT = persist.tile([128, DT, D], bf16, tag="wqT")
            wkT = persist.tile([128, CDT, D], bf16, tag="wkT")
            wvT = persist.tile([128, CDT, D], bf16, tag="wvT")
            woT = persist.tile([128, DT, D], bf16, tag="woT")
            qT = persist.tile([128, ET, M], bf16, tag="qT")
            kT = persist.tile([128, ET, LC], bf16, tag="kT")
            vA = persist.tile([128, CT, H * (HD + 1)], bf16, tag="vA")
            attnT = persist.tile([128, DT, M], bf16, tag="attnT")
            x_f32 = xf_pool.tile([128, MT, D], f32, tag="x_f32")

            # ---- load + transpose a [rows, width] DRAM tensor into
            # dest[:, dt, :] (d-major): fp32->bf16 cast via SWDGE DMA into a
            # DRAM scratch (chunked by row-halves so no single DMA convoys the
            # queue), then one xbar transpose-DMA per 128-col strip ----
            def load_transposed(src_d, rows, width, dest, after=None):
                scr = dram_pool.tile([rows, width], bf16, tag="scr")
                c = nc.gpsimd.dma_start(scr[:], src_d)
                if after is not None:
                    tile.add_dep_helper(
                        c.ins, after, reason="hold late cast behind k-path"
                    )
                t = nc.sync.dma_start_transpose(out=dest[:], in_=scr[:])
                return t.ins

            load_transposed(ctx_d, LC, CD, cT)
            k_done = load_transposed(wk_d, D, CD, wkT)

            load_transposed(wv_d, D, CD, wvT, after=k_done)
            load_transposed(x_d, M, D, xT, after=k_done)
            load_transposed(wq_d, D, D, wqT, after=k_done)

            # ---- kT[e, c] projection: lhsT = WkT, rhs = ctxT ----
            for et in range(ET):
                for cc in range(2):
                    ps = mmps.tile([128, 512], f32)
                    for cdt in range(CDT):
                        nc.tensor.matmul(
                            ps[:],
                            wkT[:, cdt, et * 128 : (et + 1) * 128],
                            cT[:, cdt, cc * 512 : (cc + 1) * 512],
                            start=(cdt == 0),
                            stop=(cdt == CDT - 1),
                        )
                    nc.any.tensor_copy(kT[:, et, cc * 512 : (cc + 1) * 512], ps[:])

            # ---- v natural [c, e] with ones column per head (augmented) ----
            for ct in range(CT):
                nc.gpsimd.memset(
                    vA[:, ct, :].rearrange("p (h w) -> p h w", w=HD + 1)[:, :, HD:],
                    1.0,
                )
                for ec in range(2):
                    ps = mmps.tile([128, 512], f32)
                    for cdt in range(CDT):
                        nc.tensor.matmul(
                            ps[:],
                            cT[:, cdt, ct * 128 : (ct + 1) * 128],
                            wvT[:, cdt, ec * 512 : (ec + 1) * 512],
                            start=(cdt == 0),
                            stop=(cdt == CDT - 1),
                        )
                    nc.any.tensor_copy(
                        vA[:, ct, :].rearrange("p (h w) -> p h w", w=HD + 1)[
                            :, ec * 8 : (ec + 1) * 8, 0:HD
                        ],
                        ps[:].rearrange("p (h w) -> p h w", w=HD),
                    )

            # ---- qT[e, m] projection: lhsT = WqT, rhs = xT ----
            for et in range(ET):
                ps = mmps.tile([128, 512], f32)
                for dt in range(DT):
                    nc.tensor.matmul(
                        ps[:],
                        wqT[:, dt, et * 128 : (et + 1) * 128],
                        xT[:, dt, :],
                        start=(dt == 0),
                        stop=(dt == DT - 1),
                    )
                nc.any.tensor_copy(qT[:, et, :], ps[:])

            load_transposed(wo_d, D, D, woT, after=k_done)

            # residual copy of x in fp32, with bo pre-folded in
            nc.sync.dma_start(x_f32[:], x_d.rearrange("(t p) d -> p t d", p=128))
            for mt in range(MT):
                nc.vector.tensor_add(x_f32[:, mt, :], x_f32[:, mt, :], bo_b[:])

            # ---- attention, two heads (one e-tile) at a time ----
            # out-projection partials over the first half of the d-contraction
            # are emitted mid-attention (after et==3) so they hide under the
            # ACT-bound softmax phase; the dt 4..7 half runs in the tail.
            opart = persist.tile([128, MT, D], f32, tag="opart")

            def emit_out_proj_half(lo, hi, into_partial):
                for mt in range(MT):
                    osb = None if into_partial else out_pool.tile(
                        [128, D], f32, tag="outsb"
                    )
                    for ec in range(2):
                        ps = mmps.tile([128, 512], f32)
                        for dt in range(lo, hi):
                            nc.tensor.matmul(
                                ps[:],
                                attnT[:, dt, mt * 128 : (mt + 1) * 128],
                                woT[:, dt, ec * 512 : (ec + 1) * 512],
                                start=(dt == lo),
                                stop=(dt == hi - 1),
                            )
                        osl_p = opart[:, mt, ec * 512 : (ec + 1) * 512]
                        if into_partial:
                            nc.vector.tensor_add(
                                osl_p, ps[:], x_f32[:, mt, ec * 512 : (ec + 1) * 512]
                            )
                        else:
                            nc.vector.tensor_add(
                                osb[:, ec * 512 : (ec + 1) * 512], ps[:], osl_p
                            )
                    if not into_partial:
                        nc.sync.dma_start(out_r[mt], osb[:])

            out_r = out_d.rearrange("(t p) d -> t p d", p=128)
            for et in range(ET):
                for half in range(2):
                    h = 2 * et + half
                    rows = slice(half * HD, (half + 1) * HD)
                    av = avps.tile([HD + 1, 512], f32)
                    for ctp in range(CT // 2):
                        sc = scps.tile([128, 1024], f32)
                        for k2 in range(2):
                            ct = 2 * ctp + k2
                            nc.tensor.matmul(
                                sc[:, k2 * 512 : (k2 + 1) * 512],
                                kT[rows, et, ct * 128 : (ct + 1) * 128],
                                qT[rows, et, :],
                                start=True,
                                stop=True,
                            )
                        pt = p_pool.tile([128, 1024], bf16, tag="p")
                        nc.scalar.activation(
                            out=pt[:], in_=sc[:], func=Exp, scale=SCALE
                        )
                        for k2 in range(2):
                            ct = 2 * ctp + k2
                            nc.tensor.matmul(
                                av[:],
                                vA[:, ct, h * (HD + 1) : (h + 1) * (HD + 1)],
                                pt[:, k2 * 512 : (k2 + 1) * 512],
                                start=(ct == 0),
                                stop=(ct == CT - 1),
                            )
                    rcp = r_pool.tile([1, 512], f32, tag="r")
                    nc.vector.reciprocal(rcp[:], av[HD : HD + 1, :])
                    rcp_b = r_pool.tile([HD, 512], f32, tag="rb")
                    nc.gpsimd.partition_broadcast(rcp_b[:], rcp[:])
                    nc.vector.tensor_mul(attnT[rows, et, :], av[0:HD, :], rcp_b[:])
            # part1 emitted after the loop: its deps clear once et<=3 heads
            # finish, so the scheduler uses it to fill PE idle slots during
            # the ACT-bound tail of attention
            emit_out_proj_half(0, 4, True)
            emit_out_proj_half(4, DT, False)

    nc.compile()
    return nc


def kernel(x, context, Wq, Wk, Wv, Wo, bo):
    global LAST_RESULT, _cached_nc
    if _cached_nc is None:
        _cached_nc = _build()
    nc = _cached_nc

    x = np.ascontiguousarray(x, dtype=np.float32)
    context = np.ascontiguousarray(context, dtype=np.float32)
    wq = np.ascontiguousarray(Wq, dtype=np.float32)
    wk = np.ascontiguousarray(Wk, dtype=np.float32)
    wv = np.ascontiguousarray(Wv, dtype=np.float32)
    wo = np.ascontiguousarray(Wo, dtype=np.float32)
    bo2 = np.ascontiguousarray(bo, dtype=np.float32).reshape(1, D)

    in_maps = []
    for c in range(NCORES):
        b = c // (NCORES // B)
        ls = (c % (NCORES // B)) * M
        in_maps.append(
            {
                "x": np.ascontiguousarray(x[b, ls : ls + M, :]),
                "ctx": context[b],
                "wq": wq,
                "wk": wk,
                "wv": wv,
                "wo": wo,
                "bo": bo2,
            }
        )

    res = run_bass_kernel_spmd(nc, in_maps, core_ids=list(range(NCORES)))
    LAST_RESULT = res

    out = np.empty((B, L, D), dtype=np.float32)
    for c in range(NCORES):
        b = c // (NCORES // B)
        ls = (c % (NCORES // B)) * M
        out[b, ls : ls + M, :] = res.results[c]["out"]
    return out

